# revision 10
# baseline (speedup 1.0000x reference)
"""Trainium2 Bass kernel for nn_Decoder_33208687133135.

Reference computation (B=2048, D=64, L=64, H=512):
    z = swapaxes(koopman, 1, 2)                    # (B, D, L)
    s = MLP_s(z); t = MLP_t(z)                     # (B, D, D), 4 layers, tanh
    ds = diag(s); dt = diag(t)                     # (B, D)
    out = (x - dt) * exp(-ds)

Only the diagonal of the (B, D, D) MLP outputs is needed, so layer 4
reduces to a per-row dot product with a single W4 column.

Layout: feature-major activations in blocks of BN=1024 rows (fixed
latent index i per block-pair).  Matmul psum tiles are [128, 1024]
(2 banks); each tanh is a single 1024-wide ACT op.  L1 z/W1 are
zero-padded to K=128 (K=64 matmuls stream at half rate on HW).

Precision (error budget rel<2e-2, numpy-sim 1.89e-2):
  L1 bf16 both MLPs.
  s-MLP: L2 f0/f1 contract K[256:512] in fp8e4 DoubleRow (h1_s packed
         via DVE copy), f2/f3 and L3 all bf16.
  t-MLP: L2 contracts K[256:512] in fp8 DR for all f (h1_t f2/f3 are
         written fp8 by ACT); L3 fully fp8 DR (2 packed K-halves).
  The fp8 K-halves of L2 get the h1 features with the smallest expected
  quantization cost via a host-side permutation of W1 cols/b1/W2 rows.

Layer 4: per-f DVE tensor_scalar products m_f = h3_f*w4_f emitted right
after each h3 tanh, then a depth-2 tensor_tensor add tree, then one
ones-matvec per (mi, nh) -> psum [1, 512] = ds/dt partials.  The
matvec+finalize of block j runs mid-L2 of block j+1 so the DVE tree
never stalls the PE.

Scheduling: L1 of block j+1 is interleaved into the L2 phase of block j
so psum-tile production stays matched to the ACT drain rate.

Sharding: latent-parallel.  Core m handles i in [8m, 8m+8) for all 2048
batches = 16384 rows = 16 blocks.  MLP weights replicated.
"""

import numpy as np
import ml_dtypes

import concourse.mybir as mybir
import concourse.tile as tile
from concourse import bacc
from concourse.bass_utils import run_bass_kernel_spmd

BF16 = mybir.dt.bfloat16
FP8 = mybir.dt.float8e4
F32 = mybir.dt.float32
_bf = ml_dtypes.bfloat16
_f8 = ml_dtypes.float8_e4m3

B, D, L, H = 2048, 64, 64, 512
NCORES = 8
IPC = D // NCORES          # latent indices per core (8)
BN = 1024                  # rows (batches) per block
BPI = B // BN              # blocks per latent index (2)
NBLK = IPC * BPI           # blocks per core (16)
NROW = IPC * B             # rows per core (16384)
S2CH = 2                   # s-MLP L2 f-tiles using the fp8 K-half

_CACHE = {}


def _build_nc():
    """Build the (single) SPMD Bass program; identical on all 8 cores."""
    nc = bacc.Bacc("TRN2", target_bir_lowering=False, debug=False,
                   num_devices=NCORES)

    Tanh = mybir.ActivationFunctionType.Tanh
    Exp = mybir.ActivationFunctionType.Exp
    DR = mybir.MatmulPerfMode.DoubleRow

    z2_d = nc.dram_tensor("z2", [128, NROW], BF16, kind="ExternalInput").ap()
    w1_d = nc.dram_tensor("w1", [128, 2, H], BF16, kind="ExternalInput").ap()
    # bf16 W2: s all 4 kc chunks, t kc 0/1 only  [p, kc, m]
    w2sbf_d = nc.dram_tensor("w2sbf", [128, 4, H], BF16,
                             kind="ExternalInput").ap()
    w2tbf_d = nc.dram_tensor("w2tbf", [128, 2, H], BF16,
                             kind="ExternalInput").ap()
    # fp8 DR K[256:512] of W2: [p, f, t, m]; k = 256 + t*128 + p
    w2s8_d = nc.dram_tensor("w2s8", [128, S2CH, 2, 128], FP8,
                            kind="ExternalInput").ap()
    w2t8_d = nc.dram_tensor("w2t8", [128, 4, 2, 128], FP8,
                            kind="ExternalInput").ap()
    # s W3 all-bf16 [p, kc, m]
    w3sbf_d = nc.dram_tensor("w3sbf", [128, 4, H], BF16,
                             kind="ExternalInput").ap()
    # t W3 fp8 DR packs: [p, pack, f, t, m]; k = pack*256 + t*128 + p
    w3t8_d = nc.dram_tensor("w3t8", [128, 2, 4, 2, 128], FP8,
                            kind="ExternalInput").ap()
    b123_d = nc.dram_tensor("b123", [128, 2, 3, 4], F32,
                            kind="ExternalInput").ap()
    # W4 columns as per-partition scalars: [p, mi, kc, i_local]
    w4s_d = nc.dram_tensor("w4s", [128, 2, 4, IPC], F32,
                           kind="ExternalInput").ap()
    one_d = nc.dram_tensor("one", [128, 1], BF16, kind="ExternalInput").ap()
    eb_d = nc.dram_tensor("eb", [1, NBLK], F32, kind="ExternalInput").ap()
    xa_d = nc.dram_tensor("xa", [1, NROW], F32, kind="ExternalInput").ap()
    out_d = nc.dram_tensor("out", [2 * NBLK, BN // 2], F32,
                           kind="ExternalOutput").ap()

    with tile.TileContext(nc) as tc:
        with (
            tc.tile_pool(name="const", bufs=1) as const,
            tc.tile_pool(name="h1p", bufs=2) as h1p,
            tc.tile_pool(name="h2p", bufs=1) as h2p,
            tc.tile_pool(name="h3p", bufs=1) as h3p,
            tc.tile_pool(name="mp", bufs=1) as mp,
            tc.tile_pool(name="fin", bufs=2) as fin,
            tc.tile_pool(name="pmm", bufs=4, space="PSUM") as pmm,
        ):
            # --- constant tiles ---
            w1c = const.tile([128, 2, H], BF16, tag="w1")
            w1_t = [w1c[:, mi] for mi in range(2)]
            bc = const.tile([128, 2, 3, 4], F32, tag="b123")
            b_t = [[bc[:, mi, ly] for ly in range(3)] for mi in range(2)]
            zbig = const.tile([128, NROW], BF16, tag="z")
            w2sbf = const.tile([128, 4, H], BF16, tag="w2sbf")
            w2tbf = const.tile([128, 2, H], BF16, tag="w2tbf")
            w2s8 = const.tile([128, S2CH, 2, 128], FP8, tag="w2s8")
            w2t8 = const.tile([128, 4, 2, 128], FP8, tag="w2t8")
            w3sbf = const.tile([128, 4, H], BF16, tag="w3sbf")
            w3t8c = const.tile([128, 2, 4, 2, 128], FP8, tag="w3t8")
            w3t8_t = [w3t8c[:, pk] for pk in range(2)]
            w4sc = const.tile([128, 2, 4, IPC], F32, tag="w4s")
            w4s_t = [w4sc[:, mi] for mi in range(2)]
            one_t = const.tile([128, 1], BF16, tag="one")
            eb_t = const.tile([1, NBLK], F32, tag="eb")
            xa_t = const.tile([1, NROW], F32, tag="xa")

            # --- DMA prologue; first block's needs first, spread queues ---
            nc.sync.dma_start(w1c[:], w1_d)
            nc.sync.dma_start(zbig[0:64, 0:BN], z2_d[0:64, 0:BN])
            nc.sync.dma_start(zbig[64:128, 0:BN], z2_d[64:128, 0:BN])
            nc.sync.dma_start(bc[:], b123_d)
            nc.sync.dma_start(one_t[:], one_d)
            nc.sync.dma_start(w2sbf[:], w2sbf_d)
            nc.sync.dma_start(w2tbf[:], w2tbf_d)
            nc.sync.dma_start(w2s8[:], w2s8_d)
            nc.sync.dma_start(w2t8[:], w2t8_d)
            nc.sync.dma_start(w3sbf[:], w3sbf_d)
            nc.sync.dma_start(w3t8c[:], w3t8_d)
            nc.sync.dma_start(w4sc[:], w4s_d)
            nc.sync.dma_start(eb_t[:], eb_d)
            nc.sync.dma_start(xa_t[:], xa_d)
            nc.sync.dma_start(zbig[:, BN:2 * BN], z2_d[:, BN:2 * BN])
            for s in range(1, 8):            # blocks 2-15
                c0, c1 = s * (NROW // 8), (s + 1) * (NROW // 8)
                nc.sync.dma_start(zbig[:, c0:c1], z2_d[:, c0:c1])

            h1s = {}     # (j, mi, f) -> bf16 h1 tile; (j, mi, 'pk') -> packed
            h2bf = {}    # f -> bf16 h2 tile (s-MLP)
            h2f8 = {}    # pack -> packed fp8 h2 tile (t-MLP)
            msum = {}    # (j, mi) -> v tile (root of the TT add tree)

            def emit_l1_group(j, mi, f):
                p = pmm.tile([128, BN], F32, tag="mm", name=f"p1_{j}_{mi}_{f}")
                for nh in range(2):
                    c0 = j * BN + nh * 512
                    nc.tensor.matmul(p[:, nh * 512:(nh + 1) * 512],
                                     w1_t[mi][:, f * 128:(f + 1) * 128],
                                     zbig[:, c0:c0 + 512],
                                     start=True, stop=True)
                bias = b_t[mi][0][:, f:f + 1]
                if mi == 1 and f >= 2:
                    # t-MLP upper half: straight to packed fp8
                    if f == 2:
                        h1s[(j, 1, 'pk')] = h1p.tile(
                            [128, 2, BN], FP8, tag="h1t8", name=f"h1t8_{j}")
                    nc.scalar.activation(h1s[(j, 1, 'pk')][:, f - 2, :], p[:],
                                         Tanh, bias=bias)
                    return
                h = h1p.tile([128, BN], BF16, tag=f"h1_{mi}_{f}",
                             name=f"h1_{j}_{mi}_{f}")
                nc.scalar.activation(h[:], p[:], Tanh, bias=bias)
                h1s[(j, mi, f)] = h
                if mi == 0 and f >= 2:
                    # s-MLP upper half: dual storage, fp8 copy via DVE
                    if f == 2:
                        h1s[(j, 0, 'pk')] = h1p.tile(
                            [128, 2, BN], FP8, tag="h1s8", name=f"h1s8_{j}")
                    nc.vector.tensor_copy(h1s[(j, 0, 'pk')][:, f - 2, :], h[:])

            def emit_l2_group(j, mi, f):
                p = pmm.tile([128, BN], F32, tag="mm", name=f"p2_{j}_{mi}_{f}")
                use_dr = (mi == 1) or (f < S2CH)
                if use_dr:
                    w8 = w2t8 if mi == 1 else w2s8
                    wbf = w2tbf if mi == 1 else w2sbf

                    def mm_dr(nh):
                        nc.tensor.matmul(
                            p[:, nh * 512:(nh + 1) * 512],
                            w8[:, f],
                            h1s[(j, mi, 'pk')][:, :, nh * 512:(nh + 1) * 512],
                            start=True, stop=False, perf_mode=DR)

                    def mm_bf(kc, nh):
                        nc.tensor.matmul(
                            p[:, nh * 512:(nh + 1) * 512],
                            wbf[:, kc, f * 128:(f + 1) * 128],
                            h1s[(j, mi, kc)][:, nh * 512:(nh + 1) * 512],
                            start=False, stop=(kc == 1))

                    mm_dr(0)
                    mm_bf(0, 0)
                    mm_dr(1)
                    mm_bf(0, 1)
                    mm_bf(1, 0)
                    mm_bf(1, 1)
                else:  # s-MLP f2/f3: all-bf16 contraction
                    for kc in range(4):
                        for nh in range(2):
                            nc.tensor.matmul(
                                p[:, nh * 512:(nh + 1) * 512],
                                w2sbf[:, kc, f * 128:(f + 1) * 128],
                                h1s[(j, 0, kc)][:, nh * 512:(nh + 1) * 512],
                                start=(kc == 0), stop=(kc == 3))
                bias = b_t[mi][1][:, f:f + 1]
                if mi == 1:
                    pk = f // 2
                    if f % 2 == 0:
                        h2f8[pk] = h2p.tile([128, 2, BN], FP8,
                                            tag=f"h2t8_{pk}",
                                            name=f"h2t8_{j}_{pk}")
                    nc.scalar.activation(h2f8[pk][:, f % 2, :], p[:], Tanh,
                                         bias=bias)
                else:
                    h = h2p.tile([128, BN], BF16, tag=f"h2_{f}",
                                 name=f"h2_{j}_{f}")
                    nc.scalar.activation(h[:], p[:], Tanh, bias=bias)
                    h2bf[f] = h

            def emit_l3_tail(j, mi, f, p):
                """tanh + the layer-4 DVE product for this f chunk."""
                il = j // BPI
                h = h3p.tile([128, BN], BF16, tag=f"h3_{mi}_{f}",
                             name=f"h3_{j}_{mi}_{f}")
                nc.scalar.activation(h[:], p[:], Tanh,
                                     bias=b_t[mi][2][:, f:f + 1])
                m = mp.tile([128, BN], BF16, tag=f"m_{mi}_{f}",
                            name=f"m_{j}_{mi}_{f}")
                nc.vector.tensor_scalar_mul(m[:], h[:],
                                            w4s_t[mi][:, f, il:il + 1])
                return m

            def emit_l3_group_s(j, f, ms):
                p = pmm.tile([128, BN], F32, tag="mm", name=f"p3_{j}_0_{f}")
                for kc in range(4):
                    for nh in range(2):
                        nc.tensor.matmul(
                            p[:, nh * 512:(nh + 1) * 512],
                            w3sbf[:, kc, f * 128:(f + 1) * 128],
                            h2bf[kc][:, nh * 512:(nh + 1) * 512],
                            start=(kc == 0), stop=(kc == 3))
                ms.append(emit_l3_tail(j, 0, f, p))

            def emit_l3_group_t(j, f, ms):
                p = pmm.tile([128, BN], F32, tag="mm", name=f"p3_{j}_1_{f}")
                # accumulate pack0 (start) then pack1 (stop) per 512-col half
                for pk in range(2):
                    for nh in range(2):
                        nc.tensor.matmul(
                            p[:, nh * 512:(nh + 1) * 512],
                            w3t8_t[pk][:, f],
                            h2f8[pk][:, :, nh * 512:(nh + 1) * 512],
                            start=(pk == 0), stop=(pk == 1), perf_mode=DR)
                ms.append(emit_l3_tail(j, 1, f, p))

            def emit_vtree(j, mi, ms):
                """Depth-2 add tree over the four m tiles -> v = ms[0]."""
                nc.vector.tensor_add(ms[0][:], ms[0][:], ms[1][:])
                nc.vector.tensor_add(ms[2][:], ms[2][:], ms[3][:])
                nc.vector.tensor_add(ms[0][:], ms[0][:], ms[2][:])
                msum[(j, mi)] = ms[0]

            def emit_finalize(j):
                """psd matvecs + (x - dt) * exp(-ds) + out DMA for block j."""
                vs = [msum.pop((j, 0)), msum.pop((j, 1))]
                for nh in range(2):
                    # both matvecs land in one pmm tile: s -> bank0 corner,
                    # t -> bank1 corner
                    pd = pmm.tile([128, BN], F32, tag="mm",
                                  name=f"pd_{j}_{nh}")
                    for mi in range(2):
                        nc.tensor.matmul(pd[0:1, mi * 512:mi * 512 + 512],
                                         one_t[:],
                                         vs[mi][:, nh * 512:(nh + 1) * 512],
                                         start=True, stop=True)
                    es = fin.tile([1, 512], F32, tag="es", name=f"es_{j}_{nh}")
                    nc.scalar.activation(es[:], pd[0:1, 0:512], Exp, scale=-1.0,
                                         bias=eb_t[:, j:j + 1])
                    tmp = fin.tile([1, 512], F32, tag="tmp", name=f"tm_{j}_{nh}")
                    c0 = j * BN + nh * 512
                    nc.vector.tensor_sub(tmp[:], xa_t[:, c0:c0 + 512],
                                         pd[0:1, 512:1024])
                    outt = fin.tile([1, 512], F32, tag="outt", name=f"ou_{j}_{nh}")
                    nc.vector.tensor_mul(outt[:], tmp[:], es[:])
                    nc.sync.dma_start(out_d[2 * j + nh:2 * j + nh + 1, :],
                                      outt[:])

            # ---- prologue: L1 of block 0 ----
            for mi in range(2):
                for f in range(4):
                    emit_l1_group(0, mi, f)

            finalize = None
            for j in range(NBLK):
                # ---- L2 phase, with L1(j+1) interleaved + finalize(j-1) ----
                l2_order = [(f, mi) for f in range(4) for mi in range(2)]
                for idx, (f, mi) in enumerate(l2_order):
                    emit_l2_group(j, mi, f)
                    if idx == 5 and finalize is not None:
                        finalize()
                        finalize = None
                    if j + 1 < NBLK:
                        emit_l1_group(j + 1, idx % 2, idx // 2)
                # drop block-j h1 references (bufs rotate by tag)
                for mi in range(2):
                    for f in range(4):
                        h1s.pop((j, mi, f), None)
                    h1s.pop((j, mi, 'pk'), None)
                # ---- L3 phase: alternate s (bf16) and t (pure DR); on the
                # last block run t first so its DVE tree (the tail
                # serializer) finishes sooner ----
                ms_s, ms_t = [], []
                last = j == NBLK - 1
                for f in range(4):
                    if last:
                        emit_l3_group_t(j, f, ms_t)
                        emit_l3_group_s(j, f, ms_s)
                    else:
                        emit_l3_group_s(j, f, ms_s)
                        emit_l3_group_t(j, f, ms_t)
                if last:
                    emit_vtree(j, 1, ms_t)
                    emit_vtree(j, 0, ms_s)
                else:
                    emit_vtree(j, 0, ms_s)
                    emit_vtree(j, 1, ms_t)
                finalize = (lambda jj=j: emit_finalize(jj))
            finalize()

    nc.compile()
    return nc


def _perm_l2(W1, b1, W2, zs):
    """Order h1 features so the fp8 K-half (256:512) gets the ones with
    the smallest expected quantization cost.  Cost per feature f:
    E[(h1_f - fp8(h1_f))^2] * ||W2[f, :]||^2, estimated on a z sample."""
    f32 = np.float32
    h1 = np.tanh(zs @ W1 + b1)
    qe = (h1 - h1.astype(_f8).astype(f32)) ** 2
    cost = qe.mean(0) * (W2 ** 2).sum(1)
    order = np.argsort(cost)              # ascending cost
    return np.concatenate([np.sort(order[256:]), np.sort(order[:256])])


def _prep_in_maps(inputs):
    """Host-side sharding: slice/cast per-core input arrays."""
    f32 = np.float32
    g = {k: np.asarray(v, f32) for k, v in inputs.items()}
    koopman, x = g["koopman"], g["x"]

    zs = np.ascontiguousarray(
        koopman[:128].transpose(0, 2, 1)).reshape(-1, L).astype(f32)
    W1p, b1p, W2p = {}, {}, {}
    for p in "st":
        W1, b1, W2 = g[p + "W1"], g[p + "b1"], g[p + "W2"]
        perm = _perm_l2(W1, b1, W2, zs)
        W1p[p], b1p[p], W2p[p] = W1[:, perm], b1[perm], W2[perm, :]

    # z2[l, i, b] = koopman[b, l, i]; bf16 once, then slice per core
    kt = np.ascontiguousarray(koopman.transpose(1, 2, 0)).astype(_bf)
    xT = np.ascontiguousarray(x.T)  # [D, B]

    # w1: [l, mi, m], zero-padded to 128 contraction rows (K=64 matmuls
    # stream at half rate on HW; K=128 with zero rows runs full speed)
    w1 = np.zeros((128, 2, H), _bf)
    w1[:L] = np.stack([W1p["s"], W1p["t"]], axis=1).astype(_bf)
    # bf16 W2 [p, kc, m]
    w2sbf = np.ascontiguousarray(
        W2p["s"].reshape(4, 128, H).transpose(1, 0, 2)).astype(_bf)
    w2tbf = np.ascontiguousarray(
        W2p["t"][:256].reshape(2, 128, H).transpose(1, 0, 2)).astype(_bf)

    # fp8 DR K[256:512] of W2: [p, f, t, m]
    def w2pack(W2, nf):
        a = W2[256:512].reshape(2, 128, 4, 128)       # [t, p, f, m]
        return np.ascontiguousarray(
            a.transpose(1, 2, 0, 3)[:, :nf]).astype(_f8)

    w2s8 = w2pack(W2p["s"], S2CH)
    w2t8 = w2pack(W2p["t"], 4)
    # s W3 bf16 [p, kc, m]
    w3sbf = np.ascontiguousarray(
        g["sW3"].reshape(4, 128, H).transpose(1, 0, 2)).astype(_bf)
    # t W3 fp8 packs [p, pack, f, t, m]; k = pack*256 + t*128 + p
    w3t8 = np.ascontiguousarray(
        g["tW3"].reshape(2, 2, 128, 4, 128).transpose(2, 0, 3, 1, 4)
    ).astype(_f8)
    w4 = np.stack([g["sW4"], g["tW4"]])  # (2, H, D) f32
    # b123: [p, mi, ly, f]; b1 permuted
    b123 = np.empty((128, 2, 3, 4), f32)
    for mi, p in enumerate("st"):
        for ly, bv in enumerate((b1p[p], g[p + "b2"], g[p + "b3"])):
            b123[:, mi, ly, :] = bv.reshape(4, 128).T
    b4s, b4t = g["sb4"], g["tb4"]
    one = np.ones((128, 1), _bf)

    in_maps = []
    for m in range(NCORES):
        i0 = m * IPC
        z2c = np.zeros((128, NROW), _bf)
        z2c[:L] = np.ascontiguousarray(
            kt[:, i0:i0 + IPC, :]).reshape(L, NROW)
        # [p, mi, kc, il]
        w4s = np.ascontiguousarray(
            w4[:, :, i0:i0 + IPC].reshape(2, 4, 128, IPC).transpose(2, 0, 1, 3)
        ).astype(f32)
        eb = np.repeat(-b4s[i0:i0 + IPC], BPI).astype(f32).reshape(1, NBLK)
        xa = (xT[i0:i0 + IPC] - b4t[i0:i0 + IPC, None]).astype(f32)
        in_maps.append({
            "z2": z2c, "w1": w1,
            "w2sbf": w2sbf, "w2tbf": w2tbf, "w2s8": w2s8, "w2t8": w2t8,
            "w3sbf": w3sbf, "w3t8": w3t8,
            "w4s": w4s, "b123": b123, "one": one, "eb": eb,
            "xa": np.ascontiguousarray(xa).reshape(1, NROW),
        })
    return in_maps


def _run(inputs, **run_kwargs):
    if "nc" not in _CACHE:
        _CACHE["nc"] = _build_nc()
    nc = _CACHE["nc"]
    in_maps = _prep_in_maps(inputs)
    res = run_bass_kernel_spmd(nc, in_maps, core_ids=list(range(NCORES)),
                               **run_kwargs)
    outT = np.empty((D, B), np.float32)
    for m in range(NCORES):
        i0 = m * IPC
        outT[i0:i0 + IPC] = np.asarray(
            res.results[m]["out"], np.float32).reshape(IPC, B)
    return np.ascontiguousarray(outT.T), res


def kernel(**inputs) -> np.ndarray:
    out, _ = _run(inputs)
    return out


# revision 11
# speedup vs baseline: 1.0069x; 1.0069x over previous
"""Trainium2 Bass kernel for nn_Decoder_33208687133135.

Reference computation (B=2048, D=64, L=64, H=512):
    z = swapaxes(koopman, 1, 2)                    # (B, D, L)
    s = MLP_s(z); t = MLP_t(z)                     # (B, D, D), 4 layers, tanh
    ds = diag(s); dt = diag(t)                     # (B, D)
    out = (x - dt) * exp(-ds)

Only the diagonal of the (B, D, D) MLP outputs is needed, so layer 4
reduces to a per-row dot product with a single W4 column.

Layout: feature-major activations in blocks of BN=1024 rows (fixed
latent index i per block-pair).  Matmul psum tiles are [128, 1024]
(2 banks); each tanh is a single 1024-wide ACT op.  L1 z/W1 are
zero-padded to K=128 (K=64 matmuls stream at half rate on HW).

Precision (error budget rel<2e-2, numpy-sim 1.89e-2):
  L1 bf16 both MLPs.
  s-MLP: L2 f0/f1 contract K[256:512] in fp8e4 DoubleRow (h1_s packed
         via DVE copy), f2/f3 and L3 all bf16.
  t-MLP: L2 contracts K[256:512] in fp8 DR for all f (h1_t f2/f3 are
         written fp8 by ACT); L3 fully fp8 DR (2 packed K-halves).
  The fp8 K-halves of L2 get the h1 features with the smallest expected
  quantization cost via a host-side permutation of W1 cols/b1/W2 rows.

Layer 4: per-f DVE tensor_scalar products m_f = h3_f*w4_f emitted right
after each h3 tanh, then a depth-2 tensor_tensor add tree, then one
ones-matvec per (mi, nh) -> psum [1, 512] = ds/dt partials.  The
matvec+finalize of block j runs mid-L2 of block j+1 so the DVE tree
never stalls the PE.

Scheduling: L1 of block j+1 is interleaved into the L2 phase of block j
so psum-tile production stays matched to the ACT drain rate.

Sharding: latent-parallel.  Core m handles i in [8m, 8m+8) for all 2048
batches = 16384 rows = 16 blocks.  MLP weights replicated.
"""

import numpy as np
import ml_dtypes

import concourse.mybir as mybir
import concourse.tile as tile
from concourse import bacc
from concourse.bass_utils import run_bass_kernel_spmd

BF16 = mybir.dt.bfloat16
FP8 = mybir.dt.float8e4
F32 = mybir.dt.float32
_bf = ml_dtypes.bfloat16
_f8 = ml_dtypes.float8_e4m3

B, D, L, H = 2048, 64, 64, 512
NCORES = 8
IPC = D // NCORES          # latent indices per core (8)
BN = 1024                  # rows (batches) per block
BPI = B // BN              # blocks per latent index (2)
NBLK = IPC * BPI           # blocks per core (16)
NROW = IPC * B             # rows per core (16384)
S2CH = 2                   # s-MLP L2 f-tiles using the fp8 K-half

_CACHE = {}


def _build_nc():
    """Build the (single) SPMD Bass program; identical on all 8 cores."""
    nc = bacc.Bacc("TRN2", target_bir_lowering=False, debug=False,
                   num_devices=NCORES)

    Tanh = mybir.ActivationFunctionType.Tanh
    Exp = mybir.ActivationFunctionType.Exp
    DR = mybir.MatmulPerfMode.DoubleRow

    z2_d = nc.dram_tensor("z2", [128, NROW], BF16, kind="ExternalInput").ap()
    w1_d = nc.dram_tensor("w1", [128, 2, H], BF16, kind="ExternalInput").ap()
    # bf16 W2: s all 4 kc chunks, t kc 0/1 only  [p, kc, m]
    w2sbf_d = nc.dram_tensor("w2sbf", [128, 4, H], BF16,
                             kind="ExternalInput").ap()
    w2tbf_d = nc.dram_tensor("w2tbf", [128, 2, H], BF16,
                             kind="ExternalInput").ap()
    # fp8 DR K[256:512] of W2: [p, f, t, m]; k = 256 + t*128 + p
    w2s8_d = nc.dram_tensor("w2s8", [128, S2CH, 2, 128], FP8,
                            kind="ExternalInput").ap()
    w2t8_d = nc.dram_tensor("w2t8", [128, 4, 2, 128], FP8,
                            kind="ExternalInput").ap()
    # s W3 all-bf16 [p, kc, m]
    w3sbf_d = nc.dram_tensor("w3sbf", [128, 4, H], BF16,
                             kind="ExternalInput").ap()
    # t W3 fp8 DR packs: [p, pack, f, t, m]; k = pack*256 + t*128 + p
    w3t8_d = nc.dram_tensor("w3t8", [128, 2, 4, 2, 128], FP8,
                            kind="ExternalInput").ap()
    b123_d = nc.dram_tensor("b123", [128, 2, 3, 4], F32,
                            kind="ExternalInput").ap()
    # W4 columns as per-partition scalars: [p, mi, kc, i_local]
    w4s_d = nc.dram_tensor("w4s", [128, 2, 4, IPC], F32,
                           kind="ExternalInput").ap()
    one_d = nc.dram_tensor("one", [128, 1], BF16, kind="ExternalInput").ap()
    eb_d = nc.dram_tensor("eb", [1, NBLK], F32, kind="ExternalInput").ap()
    xa_d = nc.dram_tensor("xa", [1, NROW], F32, kind="ExternalInput").ap()
    out_d = nc.dram_tensor("out", [2 * NBLK, BN // 2], F32,
                           kind="ExternalOutput").ap()

    with tile.TileContext(nc) as tc:
        with (
            tc.tile_pool(name="const", bufs=1) as const,
            tc.tile_pool(name="h1p", bufs=2) as h1p,
            tc.tile_pool(name="h2p", bufs=1) as h2p,
            tc.tile_pool(name="h3p", bufs=1) as h3p,
            tc.tile_pool(name="mp", bufs=1) as mp,
            tc.tile_pool(name="fin", bufs=2) as fin,
            tc.tile_pool(name="pmm", bufs=4, space="PSUM") as pmm,
        ):
            # --- constant tiles ---
            w1c = const.tile([128, 2, H], BF16, tag="w1")
            w1_t = [w1c[:, mi] for mi in range(2)]
            bc = const.tile([128, 2, 3, 4], F32, tag="b123")
            b_t = [[bc[:, mi, ly] for ly in range(3)] for mi in range(2)]
            zbig = const.tile([128, NROW], BF16, tag="z")
            w2sbf = const.tile([128, 4, H], BF16, tag="w2sbf")
            w2tbf = const.tile([128, 2, H], BF16, tag="w2tbf")
            w2s8 = const.tile([128, S2CH, 2, 128], FP8, tag="w2s8")
            w2t8 = const.tile([128, 4, 2, 128], FP8, tag="w2t8")
            w3sbf = const.tile([128, 4, H], BF16, tag="w3sbf")
            w3t8c = const.tile([128, 2, 4, 2, 128], FP8, tag="w3t8")
            w3t8_t = [w3t8c[:, pk] for pk in range(2)]
            w4sc = const.tile([128, 2, 4, IPC], F32, tag="w4s")
            w4s_t = [w4sc[:, mi] for mi in range(2)]
            one_t = const.tile([128, 1], BF16, tag="one")
            eb_t = const.tile([1, NBLK], F32, tag="eb")
            xa_t = const.tile([1, NROW], F32, tag="xa")

            # --- DMA prologue; first block's needs first, spread queues ---
            nc.sync.dma_start(w1c[:], w1_d)
            nc.sync.dma_start(zbig[0:64, 0:BN], z2_d[0:64, 0:BN])
            nc.sync.dma_start(zbig[64:128, 0:BN], z2_d[64:128, 0:BN])
            nc.sync.dma_start(bc[:], b123_d)
            nc.sync.dma_start(one_t[:], one_d)
            nc.sync.dma_start(w2sbf[:], w2sbf_d)
            nc.sync.dma_start(w2tbf[:], w2tbf_d)
            nc.sync.dma_start(w2s8[:], w2s8_d)
            nc.sync.dma_start(w2t8[:], w2t8_d)
            nc.sync.dma_start(w3sbf[:], w3sbf_d)
            nc.sync.dma_start(w3t8c[:], w3t8_d)
            nc.sync.dma_start(w4sc[:], w4s_d)
            nc.sync.dma_start(eb_t[:], eb_d)
            nc.sync.dma_start(xa_t[:], xa_d)
            nc.sync.dma_start(zbig[:, BN:2 * BN], z2_d[:, BN:2 * BN])
            for s in range(1, 8):            # blocks 2-15
                c0, c1 = s * (NROW // 8), (s + 1) * (NROW // 8)
                nc.sync.dma_start(zbig[:, c0:c1], z2_d[:, c0:c1])

            h1s = {}     # (j, mi, f) -> bf16 h1 tile; (j, mi, 'pk') -> packed
            h2bf = {}    # f -> bf16 h2 tile (s-MLP)
            h2f8 = {}    # pack -> packed fp8 h2 tile (t-MLP)
            msum = {}    # (j, mi) -> v tile (root of the TT add tree)

            def emit_l1_group(j, mi, f):
                p = pmm.tile([128, BN], F32, tag="mm", name=f"p1_{j}_{mi}_{f}")
                for nh in range(2):
                    c0 = j * BN + nh * 512
                    nc.tensor.matmul(p[:, nh * 512:(nh + 1) * 512],
                                     w1_t[mi][:, f * 128:(f + 1) * 128],
                                     zbig[:, c0:c0 + 512],
                                     start=True, stop=True)
                bias = b_t[mi][0][:, f:f + 1]
                if mi == 1 and f >= 2:
                    # t-MLP upper half: straight to packed fp8
                    if f == 2:
                        h1s[(j, 1, 'pk')] = h1p.tile(
                            [128, 2, BN], FP8, tag="h1t8", name=f"h1t8_{j}")
                    nc.scalar.activation(h1s[(j, 1, 'pk')][:, f - 2, :], p[:],
                                         Tanh, bias=bias)
                    return
                h = h1p.tile([128, BN], BF16, tag=f"h1_{mi}_{f}",
                             name=f"h1_{j}_{mi}_{f}")
                nc.scalar.activation(h[:], p[:], Tanh, bias=bias)
                h1s[(j, mi, f)] = h
                if mi == 0 and f >= 2:
                    # s-MLP upper half: dual storage, fp8 copy via DVE
                    if f == 2:
                        h1s[(j, 0, 'pk')] = h1p.tile(
                            [128, 2, BN], FP8, tag="h1s8", name=f"h1s8_{j}")
                    nc.vector.tensor_copy(h1s[(j, 0, 'pk')][:, f - 2, :], h[:])

            def emit_l2_group(j, mi, f):
                p = pmm.tile([128, BN], F32, tag="mm", name=f"p2_{j}_{mi}_{f}")
                use_dr = (mi == 1) or (f < S2CH)
                if use_dr:
                    w8 = w2t8 if mi == 1 else w2s8
                    wbf = w2tbf if mi == 1 else w2sbf

                    def mm_dr(nh):
                        nc.tensor.matmul(
                            p[:, nh * 512:(nh + 1) * 512],
                            w8[:, f],
                            h1s[(j, mi, 'pk')][:, :, nh * 512:(nh + 1) * 512],
                            start=True, stop=False, perf_mode=DR)

                    def mm_bf(kc, nh):
                        nc.tensor.matmul(
                            p[:, nh * 512:(nh + 1) * 512],
                            wbf[:, kc, f * 128:(f + 1) * 128],
                            h1s[(j, mi, kc)][:, nh * 512:(nh + 1) * 512],
                            start=False, stop=(kc == 1))

                    mm_dr(0)
                    mm_bf(0, 0)
                    mm_dr(1)
                    mm_bf(0, 1)
                    mm_bf(1, 0)
                    mm_bf(1, 1)
                else:  # s-MLP f2/f3: all-bf16 contraction
                    for kc in range(4):
                        for nh in range(2):
                            nc.tensor.matmul(
                                p[:, nh * 512:(nh + 1) * 512],
                                w2sbf[:, kc, f * 128:(f + 1) * 128],
                                h1s[(j, 0, kc)][:, nh * 512:(nh + 1) * 512],
                                start=(kc == 0), stop=(kc == 3))
                bias = b_t[mi][1][:, f:f + 1]
                if mi == 1:
                    pk = f // 2
                    if f % 2 == 0:
                        h2f8[pk] = h2p.tile([128, 2, BN], FP8,
                                            tag=f"h2t8_{pk}",
                                            name=f"h2t8_{j}_{pk}")
                    nc.scalar.activation(h2f8[pk][:, f % 2, :], p[:], Tanh,
                                         bias=bias)
                else:
                    h = h2p.tile([128, BN], BF16, tag=f"h2_{f}",
                                 name=f"h2_{j}_{f}")
                    nc.scalar.activation(h[:], p[:], Tanh, bias=bias)
                    h2bf[f] = h

            def emit_l3_tail(j, mi, f, p):
                """tanh + the layer-4 DVE product for this f chunk."""
                il = j // BPI
                h = h3p.tile([128, BN], BF16, tag=f"h3_{mi}_{f}",
                             name=f"h3_{j}_{mi}_{f}")
                nc.scalar.activation(h[:], p[:], Tanh,
                                     bias=b_t[mi][2][:, f:f + 1])
                m = mp.tile([128, BN], BF16, tag=f"m_{mi}_{f}",
                            name=f"m_{j}_{mi}_{f}")
                nc.vector.tensor_scalar_mul(m[:], h[:],
                                            w4s_t[mi][:, f, il:il + 1])
                return m

            def emit_l3_group_s(j, f, ms):
                p = pmm.tile([128, BN], F32, tag="mm", name=f"p3_{j}_0_{f}")
                for kc in range(4):
                    for nh in range(2):
                        nc.tensor.matmul(
                            p[:, nh * 512:(nh + 1) * 512],
                            w3sbf[:, kc, f * 128:(f + 1) * 128],
                            h2bf[kc][:, nh * 512:(nh + 1) * 512],
                            start=(kc == 0), stop=(kc == 3))
                ms.append(emit_l3_tail(j, 0, f, p))

            def emit_l3_group_t(j, f, ms):
                p = pmm.tile([128, BN], F32, tag="mm", name=f"p3_{j}_1_{f}")
                # accumulate pack0 (start) then pack1 (stop) per 512-col half
                for pk in range(2):
                    for nh in range(2):
                        nc.tensor.matmul(
                            p[:, nh * 512:(nh + 1) * 512],
                            w3t8_t[pk][:, f],
                            h2f8[pk][:, :, nh * 512:(nh + 1) * 512],
                            start=(pk == 0), stop=(pk == 1), perf_mode=DR)
                ms.append(emit_l3_tail(j, 1, f, p))

            def emit_vtree(j, mi, ms):
                """Depth-2 add tree over the four m tiles -> v = ms[0]."""
                nc.vector.tensor_add(ms[0][:], ms[0][:], ms[1][:])
                nc.vector.tensor_add(ms[2][:], ms[2][:], ms[3][:])
                nc.vector.tensor_add(ms[0][:], ms[0][:], ms[2][:])
                msum[(j, mi)] = ms[0]

            def emit_finalize(j):
                """psd matvecs + (x - dt) * exp(-ds) + out DMA for block j."""
                vs = [msum.pop((j, 0)), msum.pop((j, 1))]
                for nh in range(2):
                    # both matvecs land in one pmm tile: s -> bank0 corner,
                    # t -> bank1 corner
                    pd = pmm.tile([128, BN], F32, tag="mm",
                                  name=f"pd_{j}_{nh}")
                    for mi in range(2):
                        nc.tensor.matmul(pd[0:1, mi * 512:mi * 512 + 512],
                                         one_t[:],
                                         vs[mi][:, nh * 512:(nh + 1) * 512],
                                         start=True, stop=True)
                    es = fin.tile([1, 512], F32, tag="es", name=f"es_{j}_{nh}")
                    nc.scalar.activation(es[:], pd[0:1, 0:512], Exp, scale=-1.0,
                                         bias=eb_t[:, j:j + 1])
                    tmp = fin.tile([1, 512], F32, tag="tmp", name=f"tm_{j}_{nh}")
                    c0 = j * BN + nh * 512
                    nc.vector.tensor_sub(tmp[:], xa_t[:, c0:c0 + 512],
                                         pd[0:1, 512:1024])
                    outt = fin.tile([1, 512], F32, tag="outt", name=f"ou_{j}_{nh}")
                    nc.vector.tensor_mul(outt[:], tmp[:], es[:])
                    nc.sync.dma_start(out_d[2 * j + nh:2 * j + nh + 1, :],
                                      outt[:])

            # ---- prologue: L1 of block 0 ----
            for mi in range(2):
                for f in range(4):
                    emit_l1_group(0, mi, f)

            finalize = None
            for j in range(NBLK):
                # ---- L2 phase, with L1(j+1) interleaved + finalize(j-1) ----
                l2_order = [(f, mi) for f in range(4) for mi in range(2)]
                for idx, (f, mi) in enumerate(l2_order):
                    emit_l2_group(j, mi, f)
                    if idx == 3 and finalize is not None:
                        finalize()
                        finalize = None
                    if j + 1 < NBLK:
                        emit_l1_group(j + 1, idx % 2, idx // 2)
                # drop block-j h1 references (bufs rotate by tag)
                for mi in range(2):
                    for f in range(4):
                        h1s.pop((j, mi, f), None)
                    h1s.pop((j, mi, 'pk'), None)
                # ---- L3 phase: alternate s (bf16) and t (pure DR); on the
                # last block run t first so its DVE tree (the tail
                # serializer) finishes sooner ----
                ms_s, ms_t = [], []
                last = j == NBLK - 1
                for f in range(4):
                    if last:
                        emit_l3_group_t(j, f, ms_t)
                        emit_l3_group_s(j, f, ms_s)
                    else:
                        emit_l3_group_s(j, f, ms_s)
                        emit_l3_group_t(j, f, ms_t)
                if last:
                    emit_vtree(j, 1, ms_t)
                    emit_vtree(j, 0, ms_s)
                else:
                    emit_vtree(j, 0, ms_s)
                    emit_vtree(j, 1, ms_t)
                finalize = (lambda jj=j: emit_finalize(jj))
            finalize()

    nc.compile()
    return nc


def _perm_l2(W1, b1, W2, zs):
    """Order h1 features so the fp8 K-half (256:512) gets the ones with
    the smallest expected quantization cost.  Cost per feature f:
    E[(h1_f - fp8(h1_f))^2] * ||W2[f, :]||^2, estimated on a z sample."""
    f32 = np.float32
    h1 = np.tanh(zs @ W1 + b1)
    qe = (h1 - h1.astype(_f8).astype(f32)) ** 2
    cost = qe.mean(0) * (W2 ** 2).sum(1)
    order = np.argsort(cost)              # ascending cost
    return np.concatenate([np.sort(order[256:]), np.sort(order[:256])])


def _prep_in_maps(inputs):
    """Host-side sharding: slice/cast per-core input arrays."""
    f32 = np.float32
    g = {k: np.asarray(v, f32) for k, v in inputs.items()}
    koopman, x = g["koopman"], g["x"]

    zs = np.ascontiguousarray(
        koopman[:128].transpose(0, 2, 1)).reshape(-1, L).astype(f32)
    W1p, b1p, W2p = {}, {}, {}
    for p in "st":
        W1, b1, W2 = g[p + "W1"], g[p + "b1"], g[p + "W2"]
        perm = _perm_l2(W1, b1, W2, zs)
        W1p[p], b1p[p], W2p[p] = W1[:, perm], b1[perm], W2[perm, :]

    # z2[l, i, b] = koopman[b, l, i]; bf16 once, then slice per core
    kt = np.ascontiguousarray(koopman.transpose(1, 2, 0)).astype(_bf)
    xT = np.ascontiguousarray(x.T)  # [D, B]

    # w1: [l, mi, m], zero-padded to 128 contraction rows (K=64 matmuls
    # stream at half rate on HW; K=128 with zero rows runs full speed)
    w1 = np.zeros((128, 2, H), _bf)
    w1[:L] = np.stack([W1p["s"], W1p["t"]], axis=1).astype(_bf)
    # bf16 W2 [p, kc, m]
    w2sbf = np.ascontiguousarray(
        W2p["s"].reshape(4, 128, H).transpose(1, 0, 2)).astype(_bf)
    w2tbf = np.ascontiguousarray(
        W2p["t"][:256].reshape(2, 128, H).transpose(1, 0, 2)).astype(_bf)

    # fp8 DR K[256:512] of W2: [p, f, t, m]
    def w2pack(W2, nf):
        a = W2[256:512].reshape(2, 128, 4, 128)       # [t, p, f, m]
        return np.ascontiguousarray(
            a.transpose(1, 2, 0, 3)[:, :nf]).astype(_f8)

    w2s8 = w2pack(W2p["s"], S2CH)
    w2t8 = w2pack(W2p["t"], 4)
    # s W3 bf16 [p, kc, m]
    w3sbf = np.ascontiguousarray(
        g["sW3"].reshape(4, 128, H).transpose(1, 0, 2)).astype(_bf)
    # t W3 fp8 packs [p, pack, f, t, m]; k = pack*256 + t*128 + p
    w3t8 = np.ascontiguousarray(
        g["tW3"].reshape(2, 2, 128, 4, 128).transpose(2, 0, 3, 1, 4)
    ).astype(_f8)
    w4 = np.stack([g["sW4"], g["tW4"]])  # (2, H, D) f32
    # b123: [p, mi, ly, f]; b1 permuted
    b123 = np.empty((128, 2, 3, 4), f32)
    for mi, p in enumerate("st"):
        for ly, bv in enumerate((b1p[p], g[p + "b2"], g[p + "b3"])):
            b123[:, mi, ly, :] = bv.reshape(4, 128).T
    b4s, b4t = g["sb4"], g["tb4"]
    one = np.ones((128, 1), _bf)

    in_maps = []
    for m in range(NCORES):
        i0 = m * IPC
        z2c = np.zeros((128, NROW), _bf)
        z2c[:L] = np.ascontiguousarray(
            kt[:, i0:i0 + IPC, :]).reshape(L, NROW)
        # [p, mi, kc, il]
        w4s = np.ascontiguousarray(
            w4[:, :, i0:i0 + IPC].reshape(2, 4, 128, IPC).transpose(2, 0, 1, 3)
        ).astype(f32)
        eb = np.repeat(-b4s[i0:i0 + IPC], BPI).astype(f32).reshape(1, NBLK)
        xa = (xT[i0:i0 + IPC] - b4t[i0:i0 + IPC, None]).astype(f32)
        in_maps.append({
            "z2": z2c, "w1": w1,
            "w2sbf": w2sbf, "w2tbf": w2tbf, "w2s8": w2s8, "w2t8": w2t8,
            "w3sbf": w3sbf, "w3t8": w3t8,
            "w4s": w4s, "b123": b123, "one": one, "eb": eb,
            "xa": np.ascontiguousarray(xa).reshape(1, NROW),
        })
    return in_maps


def _run(inputs, **run_kwargs):
    if "nc" not in _CACHE:
        _CACHE["nc"] = _build_nc()
    nc = _CACHE["nc"]
    in_maps = _prep_in_maps(inputs)
    res = run_bass_kernel_spmd(nc, in_maps, core_ids=list(range(NCORES)),
                               **run_kwargs)
    outT = np.empty((D, B), np.float32)
    for m in range(NCORES):
        i0 = m * IPC
        outT[i0:i0 + IPC] = np.asarray(
            res.results[m]["out"], np.float32).reshape(IPC, B)
    return np.ascontiguousarray(outT.T), res


def kernel(**inputs) -> np.ndarray:
    out, _ = _run(inputs)
    return out
